# revision 22
# baseline (speedup 1.0000x reference)
"""LlamaMoE (H=2048, I=4096, E=8 experts, top-2, N=2048 tokens) on 8 trn2 cores.

Strategy: expert-parallel experts + token-parallel base MLP, combined with a
single AllToAll (no ReduceScatter).

Core c owns expert c and computes it only on the tokens routed to it (host
supplies the dispatch permutation: pre-gathered transposed activations plus
send/receive index maps, padded with OOB sentinels; all model math — router
logits, top-2 combine weights, expert MLPs, base MLP, combine — runs on
device). The base MLP is row-sharded: core c computes the full base MLP for
its own 256 token rows, so the base branch needs no cross-core reduction.

Expert output rows are scattered into an AllToAll send buffer grouped by
destination (token-home) core; one fp16 AllToAll delivers every token's two
expert rows to its home core while the base down-projection runs. The home
core computes the router (fp32) on its own tokens for the top-2 combine
weights, gathers its two contribution rows per token, scales and adds them
onto the base rows, and writes its 256-row output shard.
"""

import numpy as np

import concourse.bacc as bacc
import concourse.bass as bass
import concourse.mybir as mybir
import concourse.tile as tile
from concourse.bass_utils import run_bass_kernel_spmd
from concourse.masks import make_identity

P = 128
H = 2048
I_EXP = 4096
E = 8
NCORE = 8
NTOK = 2048
TOWN = NTOK // NCORE        # 256 own token rows per core
TOB = TOWN // P             # 2 own token blocks
KO = H // P                 # 16 contraction tiles for mm1
IC_E = I_EXP // P           # 32 expert intermediate chunks
IC_B = I_EXP // P           # 32 base chunks (full I, row-sharded base)
ICT = IC_E + IC_B           # 64 gate/up slabs
NB1 = 512                   # mm1 expert moving free dim (tokens)
HN = 512                    # mm2 moving free dim (H cols)
HNC = H // HN               # 4
WSUB = 16                   # wd sub-slab chunk count

F32 = mybir.dt.float32
F16 = mybir.dt.float16
I32 = mybir.dt.int32
AF = mybir.ActivationFunctionType
ALU = mybir.AluOpType
AXX = mybir.AxisListType.X

OOB_IDX = 1 << 20


def _chunks(total, step):
    out = []
    o = 0
    while o < total:
        out.append((o, min(step, total - o)))
        o += step
    return out


def _build(C, SLOT):
    NTC = (C + P - 1) // P  # dispatched token blocks (last may be partial)
    nc = bacc.Bacc(None)
    xeT_d = nc.dram_tensor("xeT", [P, KO, C], F16, kind="ExternalInput")
    xtO_d = nc.dram_tensor("xtO", [P, KO, TOWN], F16, kind="ExternalInput")
    xrO_d = nc.dram_tensor("xrO", [P, KO, TOWN], F32, kind="ExternalInput")
    wgu_d = nc.dram_tensor("wgu", [P, ICT, KO, 2 * P], F16, kind="ExternalInput")
    wde_d = nc.dram_tensor("wde", [P, HNC, IC_E, HN], F16, kind="ExternalInput")
    wdb_d = nc.dram_tensor("wdb", [P, HNC, IC_B, HN], F16, kind="ExternalInput")
    gw_d = nc.dram_tensor("gw", [P, KO, E], F32, kind="ExternalInput")
    dsti_d = nc.dram_tensor("dsti", [P, NTC], I32, kind="ExternalInput")
    rvi_d = nc.dram_tensor("rvi", [P, 2 * TOB], I32, kind="ExternalInput")
    cbi_d = nc.dram_tensor("cbi", [P, 2 * TOB], I32, kind="ExternalInput")
    out_d = nc.dram_tensor("out", [TOB, P, H], F16, kind="ExternalOutput")

    with tile.TileContext(nc) as tc:
        with (
            tc.tile_pool(name="persist", bufs=1) as persist,
            tc.tile_pool(name="xt", bufs=1) as xtp,
            tc.tile_pool(name="ht", bufs=1) as htp,
            tc.tile_pool(name="wgup", bufs=2) as wgup,
            tc.tile_pool(name="wdp", bufs=4) as wdp,
            tc.tile_pool(name="xk32", bufs=1) as xk32p,
            tc.tile_pool(name="tmp", bufs=2) as tmpp,
            tc.tile_pool(name="yesp", bufs=1) as yesp,
            tc.tile_pool(name="bsb", bufs=1) as bsbp,
            tc.tile_pool(name="rgp", bufs=1) as rgp,
            tc.tile_pool(name="osb", bufs=1) as osbp,
            tc.tile_pool(name="rsm", bufs=1) as rsm,
            tc.tile_pool(name="ps1", bufs=2, space="PSUM") as ps1,
            tc.tile_pool(name="ps2", bufs=2, space="PSUM") as ps2,
            tc.tile_pool(name="psr", bufs=1, space="PSUM") as psr,
            tc.tile_pool(name="dram", bufs=1, space="DRAM") as dram,
        ):
            send_dram = dram.tile([NCORE * SLOT, H], F16, tag="send", name="send")
            recv_dram = dram.tile([NCORE * SLOT, H], F16, tag="recv", name="recv")
            comb_dram = dram.tile([TOWN * E, 1], F32, tag="combd")

            # ============ mm1 expert: gate/up + silu*up on C tokens ========
            # (tiles padded to NTC*P; columns past C stay garbage and are
            # masked by OOB send indices downstream.) xeT arrives in k-chunks
            # so the first matmuls start as soon as chunk 0 lands.
            xeT = xtp.tile([P, KO, NTC * P], F16, tag="xt", name="xeT")
            for kq in range(4):
                nc.sync.dma_start(
                    xeT[:, kq * (KO // 4):(kq + 1) * (KO // 4), :C],
                    xeT_d[:, kq * (KO // 4):(kq + 1) * (KO // 4)],
                )
            gw_sb = persist.tile([P, KO, E], F32, tag="gw")
            nc.sync.dma_start(gw_sb, gw_d[:])
            dsti_sb = persist.tile([P, NTC], I32, tag="dsti")
            nc.sync.dma_start(dsti_sb, dsti_d[:])
            rvi_sb = persist.tile([P, 2 * TOB], I32, tag="rvi")
            nc.sync.dma_start(rvi_sb, rvi_d[:])
            cbi_sb = persist.tile([P, 2 * TOB], I32, tag="cbi")
            nc.sync.dma_start(cbi_sb, cbi_d[:])
            ht_e = htp.tile([P, IC_E, NTC * P], F16, tag="hte")
            for i in range(IC_E):
                slab = wgup.tile([P, KO, 2 * P], F16, tag="slab", name=f"sl{i}")
                nc.sync.dma_start(slab, wgu_d[:, i])
                for (no, nw) in _chunks(C, NB1):
                    nsl = slice(no, no + nw)
                    pg = ps1.tile([P, NB1], F32, tag="pg", name=f"pg{i}_{no}")
                    pu = ps1.tile([P, NB1], F32, tag="pu", name=f"pu{i}_{no}")
                    for k in range(KO):
                        nc.tensor.matmul(
                            pg[:, :nw], slab[:, k, 0:P], xeT[:, k, nsl],
                            start=(k == 0), stop=(k == KO - 1),
                        )
                    for k in range(KO):
                        nc.tensor.matmul(
                            pu[:, :nw], slab[:, k, P:2 * P], xeT[:, k, nsl],
                            start=(k == 0), stop=(k == KO - 1),
                        )
                    sil = tmpp.tile([P, NB1], F16, tag="sil")
                    nc.scalar.activation(sil[:, :nw], pg[:, :nw], AF.Silu)
                    nc.vector.tensor_tensor(
                        ht_e[:, i, nsl], sil[:, :nw], pu[:, :nw], ALU.mult
                    )

            # ============ router on own 256 tokens (strict fp32) ===========
            # logits^T: stationary = own x^T block [128h, 128tok], moving =
            # gw [128h, 8]; accumulate over k. One accumulation group at a
            # time per PSUM bank (start=True clears the whole bank's bits).
            zl_ps = psr.tile([P, TOB, E], F32, tag="zlps")
            for tb in range(TOB):
                xk = xk32p.tile([P, KO, P], F32, tag="xk")
                nc.sync.dma_start(xk, xrO_d[:, :, tb * P:(tb + 1) * P])
                for k in range(KO):
                    nc.tensor.matmul(
                        zl_ps[:, tb, :],
                        xk[:, k, :],
                        gw_sb[:, k, :],
                        start=(k == 0), stop=(k == KO - 1),
                    )
            zl = rsm.tile([P, TOB, E], F32, tag="zl")
            nc.vector.tensor_copy(zl, zl_ps)
            lmax = rsm.tile([P, TOB], F32, tag="lmax")
            nc.vector.reduce_max(lmax[:, :, None], zl, axis=AXX)
            nmax = rsm.tile([P, TOB], F32, tag="nmax")
            nc.vector.tensor_scalar_mul(nmax, lmax, -1.0)
            zex = rsm.tile([P, TOB, E], F32, tag="zex")
            for tb in range(TOB):
                nc.scalar.activation(
                    zex[:, tb, :], zl[:, tb, :], AF.Exp, bias=nmax[:, tb:tb + 1]
                )
            zlt = rsm.tile([P, TOB, E], F32, tag="zlt")
            nc.vector.tensor_scalar(zlt, zex, 1.0, None, op0=ALU.is_lt)
            zmk = rsm.tile([P, TOB, E], F32, tag="zmk")
            nc.vector.tensor_tensor(zmk, zex, zlt, ALU.mult)
            m2 = rsm.tile([P, TOB], F32, tag="m2")
            nc.vector.reduce_max(m2[:, :, None], zmk, axis=AXX)
            # per-expert top-2 mask and normalized weights: w_e =
            # zex_e * [zex_e >= m2] / (1 + m2)
            ge = rsm.tile([P, TOB, E], F32, tag="ge")
            nc.vector.tensor_tensor(
                ge, zex, m2[:, :, None].to_broadcast((P, TOB, E)), ALU.is_ge
            )
            s1 = rsm.tile([P, TOB], F32, tag="s1")
            nc.vector.tensor_scalar_add(s1, m2, 1.0)
            rcp = rsm.tile([P, TOB], F32, tag="rcp")
            nc.vector.reciprocal(rcp, s1)
            cw = rsm.tile([P, TOB, E], F32, tag="cw")
            nc.vector.tensor_tensor(cw, zex, ge, ALU.mult)
            cwn = rsm.tile([P, TOB, E], F32, tag="cwn")
            nc.vector.tensor_tensor(
                cwn, cw, rcp[:, :, None].to_broadcast((P, TOB, E)), ALU.mult
            )
            # store [TOWN*E, 1] with flat index (tb*128 + p)*8 + e
            nc.sync.dma_start(
                comb_dram[:].rearrange(
                    "(b p e) one -> p b (e one)", p=P, b=TOB, e=E
                ),
                cwn,
            )

            # ============ mm1 base: own 256 tokens, full I =================
            xtO = xtp.tile([P, KO, TOWN], F16, tag="xt", name="xtO")
            nc.sync.dma_start(xtO, xtO_d[:])
            ht_b = htp.tile([P, IC_B, TOWN], F16, tag="htb")
            for j in range(IC_B):
                slab = wgup.tile([P, KO, 2 * P], F16, tag="slab", name=f"slb{j}")
                nc.sync.dma_start(slab, wgu_d[:, IC_E + j])
                pg = ps1.tile([P, TOWN], F32, tag="pg", name=f"bpg{j}")
                pu = ps1.tile([P, TOWN], F32, tag="pu", name=f"bpu{j}")
                # interleave gate/up so each LDWEIGHTS hides under the
                # previous matmul (N=256 leaves no slack otherwise)
                for k in range(KO):
                    nc.tensor.matmul(
                        pg, slab[:, k, 0:P], xtO[:, k, :],
                        start=(k == 0), stop=(k == KO - 1),
                    )
                    nc.tensor.matmul(
                        pu, slab[:, k, P:2 * P], xtO[:, k, :],
                        start=(k == 0), stop=(k == KO - 1),
                    )
                sil = tmpp.tile([P, TOWN], F16, tag="sil")
                nc.scalar.activation(sil, pg, AF.Silu)
                nc.vector.tensor_tensor(ht_b[:, j, :], sil, pu, ALU.mult)

            # ============ mm2 expert (down) on dispatched tokens ===========
            yes_all = yesp.tile([P, NTC, H], F16, tag="yes")
            for cc in range(HNC):
                nsub = IC_E // WSUB
                subs = []
                for ss in range(nsub):
                    w = wdp.tile([P, WSUB, HN], F16, tag="wsl", name=f"we{cc}_{ss}")
                    nc.sync.dma_start(w, wde_d[:, cc, ss * WSUB:(ss + 1) * WSUB])
                    subs.append(w)
                for t in range(NTC):
                    py = ps2.tile([P, HN], F32, tag="py", name=f"pye{cc}_{t}")
                    for i in range(IC_E):
                        nc.tensor.matmul(
                            py, ht_e[:, i, t * P:(t + 1) * P],
                            subs[i // WSUB][:, i % WSUB, :],
                            start=(i == 0), stop=(i == IC_E - 1),
                        )
                    nc.scalar.activation(
                        yes_all[:, t, cc * HN:(cc + 1) * HN], py, AF.Copy
                    )
                    if cc == HNC - 1:
                        # block t's rows are complete: scatter into the A2A
                        # send buffer now so the collective can fire sooner
                        nc.gpsimd.indirect_dma_start(
                            out=send_dram[:],
                            out_offset=bass.IndirectOffsetOnAxis(
                                ap=dsti_sb[:, t:t + 1], axis=0
                            ),
                            in_=yes_all[:, t, :],
                            in_offset=None,
                            bounds_check=NCORE * SLOT - 1,
                            oob_is_err=False,
                        )
            # prefetch combine-weight rows (router output, ready long ago)
            # before the gpsimd queue blocks on the collective
            cbs = []
            for sidx in range(2 * TOB):
                cb = rgp.tile([P, 1], F32, tag=f"cb{sidx}", name=f"cb{sidx}")
                nc.gpsimd.indirect_dma_start(
                    out=cb[:],
                    out_offset=None,
                    in_=comb_dram[:],
                    in_offset=bass.IndirectOffsetOnAxis(
                        ap=cbi_sb[:, sidx:sidx + 1], axis=0
                    ),
                    bounds_check=TOWN * E - 1,
                    oob_is_err=False,
                )
                cbs.append(cb)
            # one fp16 AllToAll delivers rows to token-home cores; overlaps
            # the base down-projection below
            nc.gpsimd.collective_compute(
                "AllToAll",
                ALU.bypass,
                replica_groups=[list(range(NCORE))],
                ins=[send_dram[:]],
                outs=[recv_dram[:]],
            )

            # ============ mm2 base (down) on own tokens ====================
            base_sb = bsbp.tile([P, TOB, H], F16, tag="bsb")
            for cc in range(HNC):
                nsub = IC_B // WSUB
                subs = []
                for ss in range(nsub):
                    w = wdp.tile([P, WSUB, HN], F16, tag="wsl", name=f"wb{cc}_{ss}")
                    nc.sync.dma_start(w, wdb_d[:, cc, ss * WSUB:(ss + 1) * WSUB])
                    subs.append(w)
                for tb in range(TOB):
                    py = ps2.tile([P, HN], F32, tag="py", name=f"pyb{cc}_{tb}")
                    for j in range(IC_B):
                        nc.tensor.matmul(
                            py, ht_b[:, j, tb * P:(tb + 1) * P],
                            subs[j // WSUB][:, j % WSUB, :],
                            start=(j == 0), stop=(j == IC_B - 1),
                        )
                    nc.scalar.activation(
                        base_sb[:, tb, cc * HN:(cc + 1) * HN], py, AF.Copy
                    )

            # ============ receive: gather 2 rows per token, combine ========
            for tb in range(TOB):
                out_sb = osbp.tile([P, H], F16, tag="osb")
                acc = osbp.tile([P, H], F16, tag="acc")
                nc.vector.tensor_copy(out_sb, base_sb[:, tb, :])
                for j in range(2):
                    sidx = j * TOB + tb
                    rg = rgp.tile([P, H], F16, tag="rg")
                    nc.gpsimd.indirect_dma_start(
                        out=rg[:],
                        out_offset=None,
                        in_=recv_dram[:],
                        in_offset=bass.IndirectOffsetOnAxis(
                            ap=rvi_sb[:, sidx:sidx + 1], axis=0
                        ),
                        bounds_check=NCORE * SLOT - 1,
                        oob_is_err=False,
                    )
                    nc.vector.tensor_scalar_mul(acc[:], rg[:], cbs[sidx][:])
                    nc.vector.tensor_add(out=out_sb[:], in0=out_sb[:], in1=acc[:])
                nc.sync.dma_start(out_d[tb], out_sb)

    return nc


def _prep_inputs(x, gate_w, base_gate_up, base_down, expert_gate_up, expert_down):
    xf = np.ascontiguousarray(np.asarray(x, np.float32).reshape(NTOK, H))
    xT = np.ascontiguousarray(xf.reshape(NTOK, KO, P).transpose(2, 1, 0))
    xt16 = xT.astype(np.float16)
    gwf = np.asarray(gate_w, np.float32)
    gwp = np.ascontiguousarray(gwf.reshape(KO, P, E).transpose(1, 0, 2))

    # host-side dispatch: which tokens go to which expert (top-2 of logits)
    logits = xf @ gwf
    order = np.argsort(-logits, axis=1)
    top2 = order[:, :2]
    sel = [np.where((top2 == c).any(axis=1))[0].astype(np.int64) for c in range(NCORE)]
    cmax = max(len(s) for s in sel)
    C = max(P, ((cmax + 63) // 64) * 64)
    NTC = (C + P - 1) // P

    # destination grouping: rows from expert-core c to home-core h
    grp_start = np.zeros((NCORE, NCORE + 1), np.int64)
    for c in range(NCORE):
        home = sel[c] // TOWN
        for h in range(NCORE):
            grp_start[c, h + 1] = grp_start[c, h] + int((home == h).sum())
    max_len = int(
        max(grp_start[c, h + 1] - grp_start[c, h]
            for c in range(NCORE) for h in range(NCORE))
    )
    SLOT = (max_len + 3) // 4 * 4

    # per-core send index: slot s (token sel[c][s]) -> h*SLOT + pos_in_group
    dsti = []
    for c in range(NCORE):
        home = sel[c] // TOWN
        pos = np.arange(len(sel[c])) - grp_start[c, home]
        d = np.full(NTC * P, OOB_IDX, np.int64)
        d[: len(sel[c])] = home * SLOT + pos
        dsti.append(np.ascontiguousarray(d.reshape(NTC, P).T.astype(np.int32)))

    # per-core receive index: for own token t, contribution j in (0, 1):
    # expert e = top2 sorted; recv row = e*SLOT + pos of t within (e -> me)
    rvi = np.zeros((NCORE, P, 2 * TOB), np.int32)
    cbi = np.zeros((NCORE, P, 2 * TOB), np.int32)
    selpos = [dict() for _ in range(NCORE)]
    for c in range(NCORE):
        for i, t in enumerate(sel[c]):
            selpos[c][int(t)] = i
    for hme in range(NCORE):
        for tl in range(TOWN):
            t = hme * TOWN + tl
            tb, p = divmod(tl, P)
            es = np.sort(top2[t])
            for j, e in enumerate(es):
                i = selpos[e][t]
                pos = i - grp_start[e, hme]
                rvi[hme, p, j * TOB + tb] = e * SLOT + pos
                cbi[hme, p, j * TOB + tb] = tl * E + e

    SH = I_EXP  # full I for row-sharded base
    bgu = np.asarray(base_gate_up, np.float32)
    gb_ = bgu[:, :I_EXP].reshape(H, IC_B, P)
    ub_ = bgu[:, I_EXP:].reshape(H, IC_B, P)
    pb_ = np.concatenate([gb_, ub_], axis=2)  # [H, IC_B, 2P]
    bd = np.asarray(base_down, np.float32)
    wdb_p = np.ascontiguousarray(
        bd.reshape(IC_B, P, HNC, HN).transpose(1, 2, 0, 3)
    ).astype(np.float16)

    in_maps = []
    for c in range(NCORE):
        We = np.asarray(expert_gate_up[c], np.float32)
        ge_ = We[:, :I_EXP].reshape(H, IC_E, P)
        ue_ = We[:, I_EXP:].reshape(H, IC_E, P)
        pe_ = np.concatenate([ge_, ue_], axis=2)
        allp = np.concatenate([pe_, pb_], axis=1)  # [H, ICT, 2P]
        wgu_p = np.ascontiguousarray(
            allp.reshape(KO, P, ICT, 2 * P).transpose(1, 2, 0, 3)
        ).astype(np.float16)
        wde_p = np.ascontiguousarray(
            np.asarray(expert_down[c], np.float32)
            .reshape(IC_E, P, HNC, HN).transpose(1, 2, 0, 3)
        ).astype(np.float16)
        # pre-gathered transposed activations for this core's tokens
        xe = np.zeros((P, KO, C), np.float16)
        xe[:, :, : len(sel[c])] = xt16[:, :, sel[c]]
        own = slice(c * TOWN, (c + 1) * TOWN)
        in_maps.append(
            dict(
                xeT=np.ascontiguousarray(xe),
                xtO=np.ascontiguousarray(xt16[:, :, own]),
                xrO=np.ascontiguousarray(xT[:, :, own]),
                wgu=wgu_p, wde=wde_p, wdb=wdb_p, gw=gwp,
                dsti=dsti[c], rvi=rvi[c], cbi=cbi[c],
            )
        )
    return in_maps, C, SLOT


LAST_RESULTS = None


def kernel(x, gate_w, base_gate_up, base_down, expert_gate_up, expert_down):
    global LAST_RESULTS
    in_maps, C, SLOT = _prep_inputs(
        x, gate_w, base_gate_up, base_down, expert_gate_up, expert_down
    )
    nc = _build(C, SLOT)
    if not nc.is_finalized():
        nc.finalize()
    res = run_bass_kernel_spmd(nc, in_maps, core_ids=list(range(NCORE)))
    LAST_RESULTS = res
    y = np.empty((NTOK, H), np.float32)
    for c in range(NCORE):
        o = res.results[c]["out"]  # [TOB, P, H] f16
        y[c * TOWN:(c + 1) * TOWN] = o.reshape(TOWN, H).astype(np.float32)
    return y.reshape(1, NTOK, H)


if __name__ == "__main__":
    nc = _build(640, 96)
    print("build ok; instructions:",
          sum(len(b.instructions) for b in nc.main_func.blocks))
